# revision 16
# baseline (speedup 1.0000x reference)
"""Trainium2 Bass kernel for an 8x(2048,32) decoder block.

Sharding: data-parallel over batch. B=8 batch elements -> 8 NeuronCores,
one batch element per core, parameters replicated, no collectives.

Per-core layouts (t = 512*g + 128*j + p,  g,j in [0,4), p in [0,128)):
  row-major ("rm"):  tile[p, (n', d)]  with t = 128*n' + p   (n' = 4g+j)
  chunk-transposed ("ct"): tile[32*j + d, (g, p)]
LayerNorms run row-major; matmul chains run in ct / transposed layouts;
PE transposes convert between them. LayerNorm rsqrt is a Quake-style
Newton iteration entirely on VectorE, and FFN relu is a VectorE
tensor_scalar max, so ScalarE runs ONLY exp (+copies): one activation
table load for the whole kernel, and nothing ever evicts the exp table.

Attention per head h (head dim 8, heads live at partition base 32h):
  S^T[kv, q] matmuls (K=8 contraction, row-tiled 4 heads) write two
  2-bank PSUM tiles (heads 0-1 / heads 2-3). exp runs as two ScalarE
  ACTIVATEs, one per head-pair tile, so the next chunk's S matmuls for a
  pair start as soon as that pair's exp has read its banks: ScalarE (the
  critical-path engine, ~1 elem/cycle/lane) streams near-continuously.
  O'^T[(dout|den), q] += [V'_h | 1].T-matmul accumulation in PSUM
  (V'_h = V_h @ Wproj[8h:8h+8], folded host-side) is emitted one chunk
  late so it executes under the next chunk's exp. Causality: fully
  masked 128-col blocks are skipped; the diagonal block gets one
  triangular mask multiply on VectorE.
After each q-block g finishes its chunks, that block's softmax division,
residual, LN2, FFN and output DMA are emitted staggered over the next
few units so they execute under the remaining attention exp stream; only
g=3's epilogue runs exposed at the end.
"""

import ml_dtypes
import numpy as np

import concourse.bacc as bacc
import concourse.bass as bass
import concourse.mybir as mybir
import concourse.tile as tile
from concourse.bass import ts
from concourse.bass_utils import run_bass_kernel_spmd

B, T, D, H, HD = 8, 2048, 32, 4, 8
P = 128
NCORES = 8
FF = 4 * D  # 128
FP32 = mybir.dt.float32
BF16 = mybir.dt.bfloat16
U32 = mybir.dt.uint32
AF = mybir.ActivationFunctionType
ALU = mybir.AluOpType
AX = mybir.AxisListType
EPS = 1e-5
MAGIC = 0x5F3759DF

# fp32 const pack layout: [128, NF] with named column slices
_F_SLICES = {}
_NF = 0
for _name, _w in [
    ("wq", P), ("wk", P), ("ident", P),
    ("ln1g", D), ("ln1b", D), ("ln2g", D), ("ln2b", D), ("bproj", D),
    ("fb1", 1), ("fb2", 1),
]:
    _F_SLICES[_name] = (_NF, _NF + _w)
    _NF += _w
# bf16 const pack layout
_B_SLICES = {}
_NB = 0
for _name, _w in [("wvp", P), ("w1", FF), ("w2", D), ("mask", P)]:
    _B_SLICES[_name] = (_NB, _NB + _w)
    _NB += _w

_NC_CACHE = {}
PIPELINED_EPILOGUE = True
import os as _os
ACT_COPIES = _os.environ.get("ACT_COPIES", "1") == "1"
FLAT_EXP = _os.environ.get("FLAT_EXP", "1") == "1"
ACT_RELU = _os.environ.get("ACT_RELU", "0") == "1"



def _build_nc():
    nc = bacc.Bacc(
        "TRN2",
        target_bir_lowering=False,
        debug=False,
        enable_asserts=False,
        num_devices=NCORES,
    )

    d_in = {}

    def din(name, shape, dtype=FP32):
        d_in[name] = nc.dram_tensor(name, list(shape), dtype, kind="ExternalInput").ap()
        return d_in[name]

    din("x", (T, D))
    din("cf", (P, _NF))          # packed fp32 consts
    din("cb", (P, _NB), BF16)    # packed bf16 consts

    y_d = nc.dram_tensor("y", [T, D], FP32, kind="ExternalOutput").ap()

    with tile.TileContext(nc) as tc:
        _decoder_body(tc, d_in, y_d)
    nc.compile()
    return nc


def _decoder_body(tc, d_in, y_d):
    nc = tc.nc

    with (
        tc.tile_pool(name="pers", bufs=1) as pers,
        tc.tile_pool(name="work", bufs=2) as work,
        tc.tile_pool(name="psA", bufs=2, space="PSUM") as psA,
        tc.tile_pool(name="psS", bufs=1, space="PSUM") as psS,
        tc.tile_pool(name="psO", bufs=2, space="PSUM") as psO,
    ):
        # ---- input + packed constants: 3 DMAs, x first (it gates LN1) ----
        x_rm = pers.tile([P, 512], FP32)
        nc.sync.dma_start(
            x_rm.rearrange("p (n d) -> p n d", d=D),
            d_in["x"].rearrange("(n p) d -> p n d", p=P),
        )
        cf = pers.tile([P, _NF], FP32)
        nc.sync.dma_start(cf[:], d_in["cf"])
        cb = pers.tile([P, _NB], BF16)
        nc.sync.dma_start(cb[:], d_in["cb"])

        # HAM warm-up: ~5us of dense dummy matmuls right after the const
        # DMA, overlapping LN1's VectorE chain. Gets the PE to K=8/8
        # (2.4 GHz) before real matmul work; later gaps stay < ~3.4us so it
        # never re-throttles.
        for _ in range(8):
            wup = psA.tile([P, 512], FP32, tag="ps", name="wup")
            nc.tensor.matmul(
                wup[:, 0:256], lhsT=cf[:, 0:P], rhs=cf[:, 0:256],
                start=True, stop=True,
            )

        def fsl(name):
            a, b = _F_SLICES[name]
            return cf[:, a:b]

        def bsl(name):
            a, b = _B_SLICES[name]
            return cb[:, a:b]

        wq_sb = fsl("wq")        # bf16 values stored as fp32? no: fp32 weights
        wk_sb = fsl("wk")
        ident_sb = fsl("ident")
        g1_sb, b1_sb = fsl("ln1g"), fsl("ln1b")
        g2_sb, b2_sb = fsl("ln2g"), fsl("ln2b")
        bproj_sb = fsl("bproj")
        fb1_sb, fb2_sb = fsl("fb1"), fsl("fb2")
        wvp_sb = bsl("wvp")
        w1_sb = bsl("w1")
        w2_sb = bsl("w2")
        mask_sb = bsl("mask")
        eps_sb = pers.tile([P, 1], FP32)
        nc.vector.memset(eps_sb[:], EPS)

        def rsqrt_dve(v, tag):
            """v: [P, n] fp32 tile (variance+eps, in ~[0.15, 4]). In-place ->
            1/sqrt(v). Seed y0 = a + b/v (recip is an exact DVE iterative-
            divide op) + 3 Newton iterations, all on VectorE: no ScalarE
            activation tables touched, so the exp table is never evicted.
            Max rel err ~6e-6 for v in [0.15, 4]."""
            n = v.shape[1]
            y = work.tile([P, n], FP32, tag=tag + "y", name=tag + "y")
            nc.vector.reciprocal(y[:], v[:])
            nc.vector.tensor_scalar(
                out=y[:], in0=y[:], scalar1=0.37352439, scalar2=0.48221251,
                op0=ALU.mult, op1=ALU.add,
            )
            r = work.tile([P, n], FP32, tag=tag + "r", name=tag + "r")
            for _ in range(3):
                nc.vector.tensor_mul(r[:], y[:], y[:])
                nc.vector.tensor_mul(r[:], r[:], v[:])
                nc.vector.tensor_scalar(
                    out=r[:], in0=r[:], scalar1=-0.5, scalar2=1.5,
                    op0=ALU.mult, op1=ALU.add,
                )
                nc.vector.tensor_mul(y[:], y[:], r[:])
            nc.vector.tensor_copy(v[:], y[:])

        def layer_norm_rm(src3, g_sb, b_sb, out3, nblk, tag, act_sqrt=False):
            """src3/out3: [P, nblk, D] views; per-row (t) LN over d."""
            musum = work.tile([P, nblk], FP32, tag=tag + "mu", name=tag + "mu")
            nc.vector.reduce_sum(musum[:], src3, axis=AX.X)
            xc = work.tile([P, nblk * D], FP32, tag=tag + "xc", name=tag + "xc")
            xc3 = xc.rearrange("p (n d) -> p n d", d=D)
            nc.vector.scalar_tensor_tensor(
                out=xc3,
                in0=musum[:, :, None].to_broadcast((P, nblk, D)),
                scalar=-1.0 / D,
                in1=src3,
                op0=ALU.mult,
                op1=ALU.add,
            )
            sq = work.tile([P, nblk * D], FP32, tag=tag + "sq", name=tag + "sq")
            sq3 = sq.rearrange("p (n d) -> p n d", d=D)
            nc.vector.tensor_mul(sq3, xc3, xc3)
            var = work.tile([P, nblk], FP32, tag=tag + "is", name=tag + "is")
            nc.vector.reduce_sum(var[:], sq3, axis=AX.X)
            if act_sqrt:
                # prologue/tail only: ScalarE is idle there, so the sqrt
                # table load never evicts the exp table mid-stream.
                nc.scalar.activation(var[:], var[:], AF.Sqrt, bias=eps_sb[:], scale=1.0 / D)
                nc.vector.reciprocal(var[:], var[:])
            else:
                nc.vector.tensor_scalar(
                    out=var[:], in0=var[:], scalar1=1.0 / D, scalar2=EPS,
                    op0=ALU.mult, op1=ALU.add,
                )
                rsqrt_dve(var, tag)
            xn = work.tile([P, nblk * D], FP32, tag=tag + "xn", name=tag + "xn")
            xn3 = xn.rearrange("p (n d) -> p n d", d=D)
            nc.vector.tensor_mul(xn3, xc3, var[:, :, None].to_broadcast((P, nblk, D)))
            nc.vector.tensor_mul(xn3, xn3, g_sb[:, None, :].to_broadcast((P, nblk, D)))
            nc.vector.tensor_add(out3, xn3, b_sb[:, None, :].to_broadcast((P, nblk, D)))

        # ---- stage A: LN1, transpose to ct ----
        h_rm = pers.tile([P, 512], FP32)
        layer_norm_rm(
            x_rm.rearrange("p (n d) -> p n d", d=D), g1_sb, b1_sb,
            h_rm.rearrange("p (n d) -> p n d", d=D), 16, "ln1", act_sqrt=True,
        )
        h_ct_bf = pers.tile([P, 512], BF16)
        tp = psA.tile([P, 512], FP32, tag="ps", name="tp")
        for g in range(4):
            nc.tensor.transpose(tp[:, ts(g, P)], h_rm[:, ts(g, P)], ident_sb[:])
        nc.vector.tensor_copy(h_ct_bf[:], tp[:])

        # wq/wk as bf16 working copies (fp32 in the pack; cast once)
        wq_bf = pers.tile([P, P], BF16)
        nc.vector.tensor_copy(wq_bf[:], wq_sb)
        wk_bf = pers.tile([P, P], BF16)
        if ACT_COPIES:
            nc.scalar.copy(wk_bf[:], wk_sb)
        else:
            nc.vector.tensor_copy(wk_bf[:], wk_sb)

        # ---- stage B: Q^T (cols (g,j,q)), K^T (cols (j,g,q)), V' chunks ----
        qt_sb = pers.tile([P, T], BF16)   # rows 32h+hd valid; cols (g, j, q)
        kt_sb = pers.tile([P, T], BF16)   # cols (j, g, q)
        qt5 = qt_sb.rearrange("p (g j q) -> p g j q", g=4, j=4)
        for j in range(4):
            qt_ps = psA.tile([P, 512], FP32, tag="ps", name="qt_ps")
            nc.tensor.matmul(
                qt_ps[:],
                lhsT=wq_bf[ts(j, 32), :],
                rhs=h_ct_bf[ts(j, 32), :],
                start=True,
                stop=True,
                tile_position=(32 * j, 0),
            )
            # Q copies on ScalarE (idle here), K on VectorE: parallel streams.
            if ACT_COPIES:
                nc.scalar.copy(
                    qt5[:, :, j, :], qt_ps.rearrange("p (g q) -> p g q", g=4)
                )
            else:
                nc.vector.tensor_copy(
                    qt5[:, :, j, :], qt_ps.rearrange("p (g q) -> p g q", g=4)
                )
            kt_ps = psA.tile([P, 512], FP32, tag="ps", name="kt_ps")
            nc.tensor.matmul(
                kt_ps[:],
                lhsT=wk_bf[ts(j, 32), :],
                rhs=h_ct_bf[ts(j, 32), :],
                start=True,
                stop=True,
                tile_position=(32 * j, 0),
            )
            nc.vector.tensor_copy(kt_sb[:, ts(j, 512)], kt_ps[:])

        # V' augmented: per chunk c, per head h: [V'_h(32) | 1 | zeros(31)]
        # (64 cols per head so the O' matmuls initialize all 128 partitions)
        v_sb = pers.tile([P, 16 * H * 64], BF16)
        v4 = v_sb.rearrange("p (c h e) -> p c h e", c=16, h=H)
        nc.vector.memset(v4[:, :, :, 32:64], 0.0)
        nc.vector.memset(v4[:, :, :, 32], 1.0)
        for j in range(4):
            vp_ps = psA.tile([P, 512], FP32, tag="ps", name="vp_ps")
            for g in range(4):
                nc.tensor.matmul(
                    vp_ps[:, ts(g, P)],
                    lhsT=h_ct_bf[ts(j, 32), ts(g, P)],
                    rhs=wvp_sb[ts(j, 32), :],
                    start=True,
                    stop=True,
                    tile_position=(32 * j, 0),
                )
            for g in range(4):
                c = 4 * g + j
                dst = v4[:, c, :, 0:32]
                src = vp_ps[:, ts(g, P)].rearrange("p (h e) -> p h e", e=32)
                if j % 2 == 0 or not ACT_COPIES:
                    nc.vector.tensor_copy(dst, src)
                else:
                    nc.scalar.copy(dst, src)

        kt4 = kt_sb.rearrange("p (j g q) -> p j g q", j=4, g=4)

        # ---- attention state ----
        o_rm = pers.tile([P, 16 * 2 * P], FP32)  # [p', c, pair, (k, 64)]
        orm4 = o_rm.rearrange("p (c r e) -> p c r e", c=16, r=2)
        orm6 = o_rm.rearrange("p (g n r k e) -> p g n r k e", g=4, n=4, r=2, k=2)

        x1_rm = pers.tile([P, 512], FP32)
        x13 = x1_rm.rearrange("p (n d) -> p n d", d=D)
        h2_rm = pers.tile([P, 512], FP32)
        h2_ct = pers.tile([P, 512], FP32)
        h2_ct_bf = pers.tile([P, 512], BF16)
        a_sb = pers.tile([FF, T], BF16)  # relu(h2@W1+b1)^T, cols (j,g,p)
        a5 = a_sb.rearrange("p (j g q) -> p j g q", j=4, g=4)
        final_ct = pers.tile([P, 512], FP32)
        final_rm = pers.tile([P, 512], FP32)
        y3 = y_d.rearrange("(n p) d -> p n d", p=P)

        units = []
        o_tiles = {}
        for g in range(4):
            oA = psO.tile([P, 512], FP32, tag="o", name=f"oA{g}")
            oB = psO.tile([P, 512], FP32, tag="o", name=f"oB{g}")
            o_tiles[g] = (oA, oB)
            for c in range(4 * g + 4):
                units.append((g, c))

        p_slots = {}

        def emit_s_exp_mask(u):
            g, c = units[u]
            m = c - 4 * g  # >= 0: diagonal region chunk
            lo = 128 * m if m > 0 else 0
            gc, jc = c // 4, c % 4
            s01 = psS.tile([P, 1024], FP32, tag="s01", name="s01")
            s23 = psS.tile([P, 1024], FP32, tag="s23", name="s23")
            sv = {0: s01.rearrange("p (h q) -> p h q", h=2),
                  2: s23.rearrange("p (h q) -> p h q", h=2)}
            for h in range(4):
                nc.tensor.matmul(
                    sv[h & ~1][:, h & 1, :],
                    lhsT=kt4[32 * h : 32 * h + HD, jc, gc, :],
                    rhs=qt5[32 * h : 32 * h + HD, g, :, :],
                    start=True,
                    stop=True,
                    tile_position=(32 * h, 0),
                )
            p_t = work.tile([P, 4 * 512], BF16, tag="pt", name="p_t", bufs=3)
            p4 = p_t.rearrange("p (h q) -> p h q", h=H)
            if m <= 0:
                if FLAT_EXP:
                    # flat contiguous APs (lower ACTIVATE overhead)
                    nc.scalar.activation(p_t[:, 0:1024], s01[:], AF.Exp)
                    nc.scalar.activation(p_t[:, 1024:2048], s23[:], AF.Exp)
                else:
                    nc.scalar.activation(p4[:, 0:2, :], sv[0], AF.Exp)
                    nc.scalar.activation(p4[:, 2:4, :], sv[2], AF.Exp)
            else:
                nc.scalar.activation(p4[:, 0:2, lo:], sv[0][:, :, lo:], AF.Exp)
                nc.scalar.activation(p4[:, 2:4, lo:], sv[2][:, :, lo:], AF.Exp)
            if m >= 0:
                nc.vector.tensor_mul(
                    p4[:, :, lo : lo + P],
                    p4[:, :, lo : lo + P],
                    mask_sb[:, None, :].to_broadcast((P, H, P)),
                )
            p_slots[u] = p4

        def emit_o(u):
            g, c = units[u]
            m = c - 4 * g
            lo = 128 * m if m > 0 else 0
            nchunks = 4 * g + 4
            oA, oB = o_tiles[g]
            p4 = p_slots.pop(u)
            for h in range(4):
                ob = oA if h < 2 else oB
                base = 64 * (h % 2)
                nc.tensor.matmul(
                    ob[base : base + 64, lo:],
                    lhsT=v4[:, c, h, :],
                    rhs=p4[:, h, lo:],
                    start=(c == 0),
                    stop=(c == nchunks - 1),
                    skip_group_check=True,
                    tile_position=(0, 64 * (h % 2)),
                )

        def emit_o_transpose(g, pairs=(0, 1)):
            # transpose O' pairs back to row-major (q on partitions)
            for pair in pairs:
                ob = o_tiles[g][pair]
                osb = work.tile([P, 512], FP32, tag="osb", name="osb")
                nc.vector.tensor_copy(osb[:], ob[:])
                otp = psA.tile([P, 512], FP32, tag="ps", name="otp")
                for q in range(4):
                    nc.tensor.transpose(otp[:, ts(q, P)], osb[:, ts(q, P)], ident_sb[:])
                nc.vector.tensor_copy(
                    orm4[:, 4 * g : 4 * g + 4, pair, :],
                    otp.rearrange("p (q e) -> p q e", q=4),
                )

        def emit_divide_residual(g):
            # softmax divide, head-sum, +h +bproj for q-block g -> x1 rows
            dr = work.tile([P, 4 * 2 * 2], FP32, tag="dr", name="dr")
            dr4 = dr.rearrange("p (n r k) -> p n r k", n=4, r=2)
            nc.vector.reciprocal(dr4[:, :, :, :], orm6[:, g, :, :, :, 32])
            t1 = work.tile([P, 4 * 2 * 32], FP32, tag="cmb", name="t1")
            t1v = t1.rearrange("p (n r e) -> p n r e", n=4, r=2)
            nc.vector.tensor_mul(
                t1v,
                orm6[:, g, :, :, 0, 0:32],
                dr4[:, :, :, 0][:, :, :, None].to_broadcast((P, 4, 2, 32)),
            )
            t2 = work.tile([P, 4 * 2 * 32], FP32, tag="cmb", name="t2")
            t2v = t2.rearrange("p (n r e) -> p n r e", n=4, r=2)
            nc.vector.tensor_mul(
                t2v,
                orm6[:, g, :, :, 1, 0:32],
                dr4[:, :, :, 1][:, :, :, None].to_broadcast((P, 4, 2, 32)),
            )
            nc.vector.tensor_add(t1v, t1v, t2v)
            xg = x13[:, 4 * g : 4 * g + 4, :]
            nc.vector.tensor_add(xg, t1v[:, :, 0, :], t1v[:, :, 1, :])
            nc.vector.tensor_add(
                xg, xg, h_rm.rearrange("p (n d) -> p n d", d=D)[:, 4 * g : 4 * g + 4, :]
            )
            nc.vector.tensor_add(xg, xg, bproj_sb[:, None, :].to_broadcast((P, 4, D)))

        def emit_ln2_transpose(g):
            # LN2 on block g + transpose into h2_ct / h2_ct_bf
            layer_norm_rm(
                x13[:, 4 * g : 4 * g + 4, :], g2_sb, b2_sb,
                h2_rm.rearrange("p (n d) -> p n d", d=D)[:, 4 * g : 4 * g + 4, :],
                4, f"ln2_{g}", act_sqrt=(g == 3),
            )
            tpg = psA.tile([P, 512], FP32, tag="ps", name="tpg")
            nc.tensor.transpose(tpg[:, 0:P], h2_rm[:, ts(g, P)], ident_sb[:])
            nc.vector.tensor_copy(h2_ct[:, ts(g, P)], tpg[:, 0:P])
            if ACT_COPIES:
                nc.scalar.copy(h2_ct_bf[:, ts(g, P)], tpg[:, 0:P])
            else:
                nc.vector.tensor_copy(h2_ct_bf[:, ts(g, P)], tpg[:, 0:P])

        def emit_ffn_out(g):
            # FFN + final residual + transpose + output DMA for block g.
            # Each j-strip a-matmul gets its OWN psA ring tile: concurrent
            # row-tiled matmuls draining into the same PSUM bank are a fatal
            # HW error (verified); separate banks + per-j relu are safe.
            for j in range(4):
                a_ps = psA.tile([P, 512], FP32, tag="ps", name="a_ps")
                nc.tensor.matmul(
                    a_ps[:, 0:P],
                    lhsT=w1_sb[ts(j, 32), :],
                    rhs=h2_ct_bf[ts(j, 32), ts(g, P)],
                    start=True,
                    stop=True,
                    tile_position=(32 * j, 0),
                )
                # relu(a + b1) on VectorE (proven-safe tensor_scalar max)
                nc.vector.tensor_scalar(
                    out=a5[:, j, g, :],
                    in0=a_ps[:, 0:P],
                    scalar1=fb1_sb,
                    scalar2=0.0,
                    op0=ALU.add,
                    op1=ALU.max,
                )
            ff_ps = psA.tile([P, 512], FP32, tag="ps", name="ff_ps")
            for j2 in range(4):
                nc.tensor.matmul(
                    ff_ps[ts(j2, 32), ts(g, P)],
                    lhsT=w2_sb[:],
                    rhs=a5[:, j2, g, :],
                    start=True,
                    stop=True,
                    tile_position=(0, 32 * j2),
                )
            nc.vector.scalar_tensor_tensor(
                out=final_ct[:, ts(g, P)],
                in0=ff_ps[:, ts(g, P)],
                scalar=fb2_sb,
                in1=h2_ct[:, ts(g, P)],
                op0=ALU.add,
                op1=ALU.add,
            )
            ftp = psA.tile([P, 512], FP32, tag="ps", name="ftp")
            nc.tensor.transpose(ftp[:, 0:P], final_ct[:, ts(g, P)], ident_sb[:])
            nc.vector.tensor_copy(final_rm[:, ts(g, P)], ftp[:, 0:P])
            nc.sync.dma_start(
                y3[:, 4 * g : 4 * g + 4, :],
                final_rm.rearrange("p (n d) -> p n d", d=D)[:, 4 * g : 4 * g + 4, :],
            )

        # staggered epilogue emission: g's blocks land 1..4 units after its
        # last chunk so they hide under the next g's exp stream.
        g_end = {g: sum(4 * gg + 4 for gg in range(g + 1)) - 1 for g in range(4)}
        post = {}        # run after emit_o(u-1)
        post_pre = {}    # run between emit_s(u) and emit_o(u-1): pair-B
                         # transposes must precede the next g's first O matmul
        for g in range(4):
            post.setdefault(g_end[g] + 1, []).append(
                lambda g=g: emit_o_transpose(g, (0,)))
            post_pre.setdefault(g_end[g] + 2, []).append(
                lambda g=g: emit_o_transpose(g, (1,)))
            post.setdefault(g_end[g] + 3, []).append(
                lambda g=g: emit_divide_residual(g))
            post.setdefault(g_end[g] + 4, []).append(
                lambda g=g: emit_ln2_transpose(g))
            post.setdefault(g_end[g] + 5, []).append(
                lambda g=g: emit_ffn_out(g))

        nu = len(units)
        for u in range(nu):
            emit_s_exp_mask(u)
            for fn in post_pre.get(u, ()):
                fn()
            if u > 0:
                emit_o(u - 1)
            for fn in post.get(u, ()):
                fn()
        emit_o(nu - 1)
        for u in range(nu, nu + 6):
            for fn in post_pre.get(u, ()):
                fn()
            for fn in post.get(u, ()):
                fn()


def _host_consts(inputs):
    Wq = np.asarray(inputs["Wq"], np.float32)
    Wk = np.asarray(inputs["Wk"], np.float32)
    Wv = np.asarray(inputs["Wv"], np.float32)
    Wproj = np.asarray(inputs["Wproj"], np.float32)
    scale = float(HD) ** -0.5

    def pad_heads(W):  # [H, D, HD] -> [32, 128] block layout [d, 32h+hd]
        out = np.zeros((D, P), np.float32)
        for h in range(H):
            out[:, 32 * h : 32 * h + HD] = W[h]
        return out

    # V' = Wv[h] @ Wproj[8h:8h+8]  -> [d, 32h+dout]
    wvp = np.zeros((D, P), np.float32)
    for h in range(H):
        wvp[:, 32 * h : 32 * h + 32] = Wv[h] @ Wproj[HD * h : HD * h + HD]

    cf = np.zeros((P, _NF), np.float32)

    def put_f(name, val):
        a, b = _F_SLICES[name]
        cf[:, a:b] = val

    put_f("wq", np.tile(pad_heads(Wq * scale), (4, 1)))
    put_f("wk", np.tile(pad_heads(Wk), (4, 1)))
    put_f("ident", np.eye(P, dtype=np.float32))
    put_f("ln1g", np.asarray(inputs["ln1_g"], np.float32)[None, :])
    put_f("ln1b", np.asarray(inputs["ln1_b"], np.float32)[None, :])
    put_f("ln2g", np.asarray(inputs["ln2_g"], np.float32)[None, :])
    put_f("ln2b", np.asarray(inputs["ln2_b"], np.float32)[None, :])
    put_f("bproj", np.asarray(inputs["bproj"], np.float32)[None, :])
    put_f("fb1", np.asarray(inputs["b1"], np.float32).reshape(FF, 1))
    put_f("fb2", np.tile(np.asarray(inputs["b2"], np.float32), 4).reshape(P, 1))

    cbv = np.zeros((P, _NB), np.float32)

    def put_b(name, val):
        a, b = _B_SLICES[name]
        cbv[:, a:b] = val

    put_b("wvp", np.tile(wvp, (4, 1)))
    put_b("w1", np.tile(np.asarray(inputs["W1"], np.float32), (4, 1)))
    put_b("w2", np.asarray(inputs["W2"], np.float32))
    put_b("mask", np.triu(np.ones((P, P), np.float32)))

    return {
        "cf": np.ascontiguousarray(cf),
        "cb": np.ascontiguousarray(cbv.astype(ml_dtypes.bfloat16)),
    }


def _get_nc():
    if "nc" not in _NC_CACHE:
        _NC_CACHE["nc"] = _build_nc()
    return _NC_CACHE["nc"]


def kernel(**inputs):
    x = np.asarray(inputs["x"], np.float32)
    consts = _host_consts(inputs)
    nc = _get_nc()
    in_maps = []
    for b in range(B):
        m = dict(consts)
        m["x"] = np.ascontiguousarray(x[b])
        in_maps.append(m)
    res = run_bass_kernel_spmd(nc, in_maps, core_ids=list(range(NCORES)))
    out = np.stack([r["y"] for r in res.results], axis=0)
    return out.astype(np.float32)


def kernel_traced(**inputs):
    """Like kernel() but requests an NTFF trace; returns (out, BassKernelResults)."""
    x = np.asarray(inputs["x"], np.float32)
    consts = _host_consts(inputs)
    nc = _get_nc()
    in_maps = []
    for b in range(B):
        m = dict(consts)
        m["x"] = np.ascontiguousarray(x[b])
        in_maps.append(m)
    res = run_bass_kernel_spmd(nc, in_maps, core_ids=list(range(NCORES)), trace=True)
    out = np.stack([r["y"] for r in res.results], axis=0)
    return out.astype(np.float32), res
